# revision 10
# baseline (speedup 1.0000x reference)
"""Trainium2 Bass kernel for causal multi-head attention (B=4, T=2048, C=1024, H=16).

Sharding: tensor-parallel over heads x batch. 8 cores = 4 batches x 2 head-halves.
Each core computes, for its batch b and its 8 heads:
  qkv projection -> causal attention -> output projection partial (rows of w_proj)
Host gathers by summing the two half-partials per batch (the "all-reduce").

Per-core layouts (all fp32 in memory; matmuls run as float32r = FP22-truncated,
full PE rate at moving-dim >= 256):
  xT  [C=1024, T=2048]   x[b] transposed (host-side) so contraction dim c sits on
                         SBUF partitions for the projections.
  Q^T, K^T as [j=512, T] (4 partition-tiles of 2 heads each) -> scores are computed
  transposed: S^T[k, q] = sum_d K^T[d,k] Q^T[d,q], so softmax's sum is a matmul
  (ones column folded into V) and P^T feeds the PV matmul with no transposes.
  V as [t, j] with an interleaved ones column per head: tile [128, 8, 65].
  exp() has no max-subtraction: scores are ~N(0,1) for these inputs (|S|<~8).
"""

import sys

for _p in ("/opt/trn_rl_repo",):
    if _p not in sys.path:
        sys.path.insert(0, _p)

import numpy as np

import concourse.bass as bass
import concourse.mybir as mybir
import concourse.tile as tile
from concourse import bacc
from concourse.bass import ts
from concourse.bass_utils import run_bass_kernel_spmd

B, T, C, H, D = 4, 2048, 1024, 16, 64
NCORES = 8
JC = 512  # channels per core (8 heads x 64)
HL = 8  # heads per core
CT = C // 128  # 8 contraction tiles
TT = T // 128  # 16 t(=k) tiles
TCH = 512  # projection t-chunk
F32 = mybir.dt.float32
F32R = mybir.dt.float32r
EXP = mybir.ActivationFunctionType.Exp
ADD = mybir.AluOpType.add
MULT = mybir.AluOpType.mult


def _r(ap):
    return ap.bitcast(F32R)


def _trace(nc, tc, io):
    xT, wq, wk, wv, wp, bq, bk, bv, bp, tri, onesd, out = io

    with (
        tc.tile_pool(name="consts", bufs=1) as consts,
        tc.tile_pool(name="qk", bufs=1) as qk_pool,
        tc.tile_pool(name="vp", bufs=1) as v_pool,
    ):
        tri_sb = consts.tile([128, 128], F32, tag="tri")
        nc.sync.dma_start(out=tri_sb, in_=tri)
        bq_sb = consts.tile([128, 4], F32, tag="bq")
        nc.sync.dma_start(out=bq_sb, in_=bq.rearrange("(jt p) -> p jt", p=128))
        bk_sb = consts.tile([128, 4], F32, tag="bk")
        nc.sync.dma_start(out=bk_sb, in_=bk.rearrange("(jt p) -> p jt", p=128))
        bv_sb = consts.tile([128, JC], F32, tag="bv")
        nc.sync.dma_start(out=bv_sb, in_=bv.unsqueeze(0).to_broadcast([128, JC]))
        bp_sb = consts.tile([128, C], F32, tag="bp")
        nc.sync.dma_start(out=bp_sb, in_=bp.unsqueeze(0).to_broadcast([128, C]))

        q_sb = [qk_pool.tile([128, T], F32R, tag=f"q{jt}", name=f"q{jt}") for jt in range(4)]
        k_sb = [qk_pool.tile([128, T], F32R, tag=f"k{jt}", name=f"k{jt}") for jt in range(4)]
        v_sb = [v_pool.tile([128, HL, 65], F32R, tag=f"v{tt}", name=f"v{tt}") for tt in range(TT)]

        # ---- Phase 1: projections --------------------------------------
        with (
            tc.tile_pool(name="wat", bufs=1) as w_pool,
            tc.tile_pool(name="xt", bufs=2) as xt_pool,
            tc.tile_pool(name="pps", bufs=6, space="PSUM") as ppsum,
        ):
            wq_sb = w_pool.tile([128, CT, JC], F32R, tag="wq")
            nc.sync.dma_start(out=wq_sb, in_=wq.rearrange("(ct p) j -> p ct j", p=128))
            wk_sb = w_pool.tile([128, CT, JC], F32R, tag="wk")
            nc.sync.dma_start(out=wk_sb, in_=wk.rearrange("(ct p) j -> p ct j", p=128))
            wv_sb = w_pool.tile([128, CT, JC], F32R, tag="wv")
            nc.sync.dma_start(out=wv_sb, in_=wv.rearrange("(ct p) j -> p ct j", p=128))

            xT_r = xT.rearrange("(ct p) t -> p ct t", p=128)
            for tcn in range(T // TCH):
                xt_t = xt_pool.tile([128, CT, TCH], F32R, tag="xt")
                nc.sync.dma_start(out=xt_t, in_=xT_r[:, :, ts(tcn, TCH)])
                for wsb, bsb, dst in ((wq_sb, bq_sb, q_sb), (wk_sb, bk_sb, k_sb)):
                    for jt in range(4):
                        ps = ppsum.tile([128, TCH], F32, tag="pp")
                        for ct in range(CT):
                            nc.tensor.matmul(
                                ps,
                                lhsT=_r(wsb[:, ct, ts(jt, 128)]),
                                rhs=_r(xt_t[:, ct, :]),
                                start=(ct == 0),
                                stop=(ct == CT - 1),
                            )
                        nc.vector.tensor_scalar_add(
                            out=dst[jt][:, ts(tcn, TCH)],
                            in0=ps,
                            scalar1=bsb[:, jt : jt + 1],
                        )
                for sub in range(TCH // 128):
                    tt = tcn * (TCH // 128) + sub
                    ps = ppsum.tile([128, JC], F32, tag="pp")
                    for ct in range(CT):
                        nc.tensor.matmul(
                            ps,
                            lhsT=_r(xt_t[:, ct, ts(sub, 128)]),
                            rhs=_r(wv_sb[:, ct, :]),
                            start=(ct == 0),
                            stop=(ct == CT - 1),
                        )
                    nc.vector.tensor_tensor(
                        out=v_sb[tt][:, :, 0:64],
                        in0=ps.rearrange("p (h d) -> p h d", h=HL),
                        in1=bv_sb.rearrange("p (h d) -> p h d", h=HL),
                        op=ADD,
                    )
                    nc.sync.dma_start(
                        out=v_sb[tt][:, :, 64:65],
                        in_=onesd.unsqueeze(0).unsqueeze(2)[:, 0:HL, :].to_broadcast([128, HL, 1]),
                    )

        # ---- Phase 2: attention per head -------------------------------
        with (
            tc.tile_pool(name="yp", bufs=1) as y_pool,
            tc.tile_pool(name="wpp", bufs=1) as wp_pool,
        ):
            y_sb = [y_pool.tile([128, T], F32R, tag=f"y{jt}", name=f"y{jt}") for jt in range(4)]
            wp_sb = wp_pool.tile([128, 4, C], F32R, tag="wp")
            nc.sync.dma_start(out=wp_sb, in_=wp.rearrange("(jt p) c -> p jt c", p=128))
            _attn_and_oproj(
                nc, tc, q_sb, k_sb, v_sb, y_sb, wp_sb, tri_sb, bp_sb, out
            )


def _attn_and_oproj(nc, tc, q_sb, k_sb, v_sb, y_sb, wp_sb, tri_sb, bp_sb, out):
    if True:
        with (
            tc.tile_pool(name="pt", bufs=2) as pt_pool,
            tc.tile_pool(name="rd", bufs=1) as rd_pool,
            tc.tile_pool(name="dsc", bufs=2, space="DRAM") as d_pool,
            tc.tile_pool(name="sc", bufs=2, space="PSUM") as sc_pool,
            tc.tile_pool(name="pv", bufs=1, space="PSUM") as pv_pool,
        ):
            for h in range(HL):
                jt, hrow = h // 2, 64 * (h % 2)
                pvps = pv_pool.tile([128, T], F32, tag="pv")
                for kt in range(TT):
                    ext = T - 128 * kt
                    pt_t = pt_pool.tile([128, 2048], F32R, tag="pt")
                    for s in range((ext + 1023) // 1024):
                        w = min(1024, ext - 1024 * s)
                        scps = sc_pool.tile([128, 1024], F32, tag="sc")
                        for q5 in range(0, w, 512):
                            w5 = min(512, w - q5)
                            qabs = 128 * kt + 1024 * s + q5
                            nc.tensor.matmul(
                                scps[:, q5 : q5 + w5],
                                lhsT=_r(k_sb[jt][hrow : hrow + 64, ts(kt, 128)]),
                                rhs=_r(q_sb[jt][hrow : hrow + 64, qabs : qabs + w5]),
                                start=True,
                                stop=True,
                            )
                        nc.scalar.activation(
                            out=pt_t[:, 1024 * s : 1024 * s + w],
                            in_=scps[:, 0:w],
                            func=EXP,
                            scale=0.125,
                        )
                    # causal mask of the diagonal 128x128 block
                    nc.vector.tensor_mul(pt_t[:, 0:128], pt_t[:, 0:128], tri_sb)
                    for qb in range(kt // 4, 4):
                        lo = max(128 * kt, 512 * qb)
                        hi = 512 * (qb + 1)
                        rel = lo - 128 * kt
                        nc.tensor.matmul(
                            pvps[0:65, lo:hi],
                            lhsT=_r(v_sb[kt][:, h, :]),
                            rhs=_r(pt_t[:, rel : rel + hi - lo]),
                            start=(kt == 0),
                            stop=(kt == 4 * qb + 3),
                        )
                rden = rd_pool.tile([1, T], F32, tag="rden")
                nc.vector.reciprocal(out=rden, in_=pvps[64:65, :])
                dscr = d_pool.tile([T], F32, tag="dscr")
                nc.sync.dma_start(out=dscr.unsqueeze(0), in_=rden)
                rdb = rd_pool.tile([64, T], F32, tag="rdb")
                nc.sync.dma_start(out=rdb, in_=dscr.unsqueeze(0).to_broadcast([64, T]))
                nc.vector.tensor_tensor(
                    out=y_sb[jt][hrow : hrow + 64, :],
                    in0=pvps[0:64, :],
                    in1=rdb,
                    op=MULT,
                )

        # ---- Phase 3: output projection --------------------------------
        with (
            tc.tile_pool(name="osb", bufs=3) as o_pool,
            tc.tile_pool(name="ops", bufs=4, space="PSUM") as o_psum,
        ):
            for tt in range(TT):
                ot = o_pool.tile([128, C], F32, tag="o")
                for ch in range(2):
                    ps = o_psum.tile([128, 512], F32, tag="op")
                    for jt in range(4):
                        nc.tensor.matmul(
                            ps,
                            lhsT=_r(y_sb[jt][:, ts(tt, 128)]),
                            rhs=_r(wp_sb[:, jt, ts(ch, 512)]),
                            start=(jt == 0),
                            stop=(jt == 3),
                        )
                    nc.vector.tensor_tensor(
                        out=ot[:, ts(ch, 512)],
                        in0=ps,
                        in1=bp_sb[:, ts(ch, 512)],
                        op=ADD,
                    )
                nc.sync.dma_start(out=out[ts(tt, 128), :], in_=ot)


_CACHE = {}


def build_nc():
    if "nc" in _CACHE:
        return _CACHE["nc"]
    nc = bacc.Bacc(
        "TRN2",
        target_bir_lowering=False,
        debug=False,
        enable_asserts=False,
        num_devices=NCORES,
    )
    io = (
        nc.dram_tensor("xT", [C, T], F32R, kind="ExternalInput").ap(),
        nc.dram_tensor("wq", [C, JC], F32R, kind="ExternalInput").ap(),
        nc.dram_tensor("wk", [C, JC], F32R, kind="ExternalInput").ap(),
        nc.dram_tensor("wv", [C, JC], F32R, kind="ExternalInput").ap(),
        nc.dram_tensor("wp", [JC, C], F32R, kind="ExternalInput").ap(),
        nc.dram_tensor("bq", [JC], F32, kind="ExternalInput").ap(),
        nc.dram_tensor("bk", [JC], F32, kind="ExternalInput").ap(),
        nc.dram_tensor("bv", [JC], F32, kind="ExternalInput").ap(),
        nc.dram_tensor("bp", [C], F32, kind="ExternalInput").ap(),
        nc.dram_tensor("tri", [128, 128], F32, kind="ExternalInput").ap(),
        nc.dram_tensor("onesd", [64], F32R, kind="ExternalInput").ap(),
        nc.dram_tensor("out", [T, C], F32, kind="ExternalOutput").ap(),
    )
    with tile.TileContext(nc) as tc:
        _trace(nc, tc, io)
    nc.compile()
    _CACHE["nc"] = nc
    return nc


def make_in_maps(x, w_attn, b_attn, w_proj, b_proj):
    tri = np.triu(np.ones((128, 128), dtype=np.float32))
    zeros_c = np.zeros(C, dtype=np.float32)
    in_maps = []
    for core in range(NCORES):
        b, hh = core // 2, core % 2
        j0 = JC * hh
        in_maps.append(
            {
                "xT": np.ascontiguousarray(x[b].T).astype(np.float32, copy=False),
                "wq": np.ascontiguousarray(w_attn[:, j0 : j0 + JC]),
                "wk": np.ascontiguousarray(w_attn[:, C + j0 : C + j0 + JC]),
                "wv": np.ascontiguousarray(w_attn[:, 2 * C + j0 : 2 * C + j0 + JC]),
                "wp": np.ascontiguousarray(w_proj[j0 : j0 + JC, :]),
                "bq": np.ascontiguousarray(b_attn[j0 : j0 + JC]),
                "bk": np.ascontiguousarray(b_attn[C + j0 : C + j0 + JC]),
                "bv": np.ascontiguousarray(b_attn[2 * C + j0 : 2 * C + j0 + JC]),
                "bp": (b_proj.astype(np.float32) if hh == 0 else zeros_c),
                "tri": tri,
                "onesd": np.ones(64, dtype=np.float32),
            }
        )
    return in_maps


def gather(parts):
    out = np.empty((B, T, C), dtype=np.float32)
    for b in range(B):
        out[b] = parts[2 * b]["out"] + parts[2 * b + 1]["out"]
    return out


def kernel(x, w_attn, b_attn, w_proj, b_proj):
    x = np.asarray(x, dtype=np.float32)
    w_attn = np.asarray(w_attn, dtype=np.float32)
    b_attn = np.asarray(b_attn, dtype=np.float32)
    w_proj = np.asarray(w_proj, dtype=np.float32)
    b_proj = np.asarray(b_proj, dtype=np.float32)
    nc = build_nc()
    in_maps = make_in_maps(x, w_attn, b_attn, w_proj, b_proj)
    res = run_bass_kernel_spmd(nc, in_maps, core_ids=list(range(NCORES)))
    return gather(res.results)


if __name__ == "__main__":
    rng = np.random.default_rng(0)
    x = rng.standard_normal((B, T, C), dtype=np.float32)
    w_attn = rng.standard_normal((C, 3 * C), dtype=np.float32) / np.sqrt(C)
    b_attn = np.zeros(3 * C, np.float32)
    w_proj = rng.standard_normal((C, C), dtype=np.float32) / np.sqrt(C)
    b_proj = np.zeros(C, np.float32)
    out = kernel(x, w_attn, b_attn, w_proj, b_proj)
    print(out.shape, out.dtype, np.abs(out).mean())


# revision 12
# speedup vs baseline: 11596.4748x; 11596.4748x over previous
"""Trainium2 Bass kernel for causal multi-head attention (B=4, T=2048, C=1024, H=16).

Sharding: tensor-parallel over heads x batch. 8 cores = 4 batches x 2 head-halves.
Each core computes, for its batch b and its 8 heads:
  qkv projection -> causal attention -> output projection partial (rows of w_proj)
Host gathers by summing the two half-partials per batch (the "all-reduce").

Per-core layouts (all fp32 in memory; matmuls run as float32r = FP22-truncated,
full PE rate at moving-dim >= 256):
  xT  [C=1024, T=2048]   x[b] transposed (host-side) so contraction dim c sits on
                         SBUF partitions for the projections.
  Q^T, K^T as [j=512, T] (4 partition-tiles of 2 heads each) -> scores are computed
  transposed: S^T[k, q] = sum_d K^T[d,k] Q^T[d,q], so softmax's sum is a matmul
  (ones column folded into V) and P^T feeds the PV matmul with no transposes.
  V as [t, j] with an interleaved ones column per head: tile [128, 8, 65].
  exp() has no max-subtraction: scores are ~N(0,1) for these inputs (|S|<~8).
"""

import sys

for _p in ("/opt/trn_rl_repo",):
    if _p not in sys.path:
        sys.path.insert(0, _p)

import numpy as np

import concourse.bass as bass
import concourse.mybir as mybir
import concourse.tile as tile
from concourse import bacc
from concourse.bass import ts
from concourse.bass_utils import run_bass_kernel_spmd

B, T, C, H, D = 4, 2048, 1024, 16, 64
NCORES = 8
JC = 512  # channels per core (8 heads x 64)
HL = 8  # heads per core
CT = C // 128  # 8 contraction tiles
TT = T // 128  # 16 t(=k) tiles
TCH = 512  # projection t-chunk
F32 = mybir.dt.float32
F32R = mybir.dt.float32r
BF16 = mybir.dt.bfloat16
EXP = mybir.ActivationFunctionType.Exp
ADD = mybir.AluOpType.add
MULT = mybir.AluOpType.mult


def _r(ap):
    return ap.bitcast(F32R)


def _trace(nc, tc, io):
    xT, wq, wk, wv, wp, bq, bk, bv, bp, tri, out = io

    with (
        tc.tile_pool(name="consts", bufs=1) as consts,
        tc.tile_pool(name="qk", bufs=1) as qk_pool,
        tc.tile_pool(name="vp", bufs=1) as v_pool,
    ):
        tri_sb = consts.tile([128, 128], BF16, tag="tri")
        nc.sync.dma_start(out=tri_sb, in_=tri)
        bq_sb = consts.tile([128, 4], F32, tag="bq")
        nc.sync.dma_start(out=bq_sb, in_=bq.rearrange("(jt p) -> p jt", p=128))
        bk_sb = consts.tile([128, 4], F32, tag="bk")
        nc.sync.dma_start(out=bk_sb, in_=bk.rearrange("(jt p) -> p jt", p=128))
        bv_sb = consts.tile([128, JC], F32, tag="bv")
        nc.sync.dma_start(out=bv_sb, in_=bv.unsqueeze(0).to_broadcast([128, JC]))
        bp_sb = consts.tile([128, C], F32, tag="bp")
        nc.sync.dma_start(out=bp_sb, in_=bp.unsqueeze(0).to_broadcast([128, C]))

        q_sb = [qk_pool.tile([128, T], BF16, tag=f"q{jt}", name=f"q{jt}") for jt in range(4)]
        k_sb = [qk_pool.tile([128, T], BF16, tag=f"k{jt}", name=f"k{jt}") for jt in range(4)]
        v_sb = [v_pool.tile([128, HL, 65], BF16, tag=f"v{tt}", name=f"v{tt}") for tt in range(TT)]

        # ---- Phase 1: projections --------------------------------------
        with (
            tc.tile_pool(name="wat", bufs=1) as w_pool,
            tc.tile_pool(name="xt", bufs=2) as xt_pool,
            tc.tile_pool(name="pps", bufs=6, space="PSUM") as ppsum,
        ):
            wq_sb = w_pool.tile([128, CT, JC], F32R, tag="wq")
            nc.sync.dma_start(out=wq_sb, in_=wq.rearrange("(ct p) j -> p ct j", p=128))
            wk_sb = w_pool.tile([128, CT, JC], F32R, tag="wk")
            nc.sync.dma_start(out=wk_sb, in_=wk.rearrange("(ct p) j -> p ct j", p=128))
            wv_sb = w_pool.tile([128, CT, JC], F32R, tag="wv")
            nc.sync.dma_start(out=wv_sb, in_=wv.rearrange("(ct p) j -> p ct j", p=128))

            xT_r = xT.rearrange("(ct p) t -> p ct t", p=128)
            for tcn in range(T // TCH):
                xt_t = xt_pool.tile([128, CT, TCH], F32R, tag="xt")
                nc.sync.dma_start(out=xt_t, in_=xT_r[:, :, ts(tcn, TCH)])
                for wsb, bsb, dst in ((wq_sb, bq_sb, q_sb), (wk_sb, bk_sb, k_sb)):
                    for jt in range(4):
                        ps = ppsum.tile([128, TCH], F32, tag="pp")
                        for ct in range(CT):
                            nc.tensor.matmul(
                                ps,
                                lhsT=_r(wsb[:, ct, ts(jt, 128)]),
                                rhs=_r(xt_t[:, ct, :]),
                                start=(ct == 0),
                                stop=(ct == CT - 1),
                            )
                        nc.vector.tensor_scalar_add(
                            out=dst[jt][:, ts(tcn, TCH)],
                            in0=ps,
                            scalar1=bsb[:, jt : jt + 1],
                        )
                for sub in range(TCH // 128):
                    tt = tcn * (TCH // 128) + sub
                    ps = ppsum.tile([128, JC], F32, tag="pp")
                    for ct in range(CT):
                        nc.tensor.matmul(
                            ps,
                            lhsT=_r(xt_t[:, ct, ts(sub, 128)]),
                            rhs=_r(wv_sb[:, ct, :]),
                            start=(ct == 0),
                            stop=(ct == CT - 1),
                        )
                    nc.vector.memset(v_sb[tt], 1.0)
                    nc.vector.tensor_tensor(
                        out=v_sb[tt][:, :, 0:64],
                        in0=ps.rearrange("p (h d) -> p h d", h=HL),
                        in1=bv_sb.rearrange("p (h d) -> p h d", h=HL),
                        op=ADD,
                    )

        # ---- Phase 2: attention per head -------------------------------
        with (
            tc.tile_pool(name="yp", bufs=1) as y_pool,
            tc.tile_pool(name="wpp", bufs=1) as wp_pool,
        ):
            y_sb = [y_pool.tile([128, T], F32R, tag=f"y{jt}", name=f"y{jt}") for jt in range(4)]
            wp_sb = wp_pool.tile([128, 4, C], F32R, tag="wp")
            nc.sync.dma_start(out=wp_sb, in_=wp.rearrange("(jt p) c -> p jt c", p=128))
            _attn_and_oproj(
                nc, tc, q_sb, k_sb, v_sb, y_sb, wp_sb, tri_sb, bp_sb, out
            )


def _attn_and_oproj(nc, tc, q_sb, k_sb, v_sb, y_sb, wp_sb, tri_sb, bp_sb, out):
    if True:
        with (
            tc.tile_pool(name="pt", bufs=3) as pt_pool,
            tc.tile_pool(name="rd", bufs=1) as rd_pool,
            tc.tile_pool(name="dsc", bufs=2, space="DRAM") as d_pool,
            tc.tile_pool(name="sc", bufs=2, space="PSUM") as sc_pool,
            tc.tile_pool(name="pv", bufs=1, space="PSUM") as pv_pool,
        ):
            for h in range(HL):
                jt, hrow = h // 2, 64 * (h % 2)
                pvps = pv_pool.tile([128, T], F32, tag="pv")
                for kt in range(TT):
                    ext = T - 128 * kt
                    pt_t = pt_pool.tile([128, 2048], BF16, tag="pt")
                    for s in range((ext + 1023) // 1024):
                        w = min(1024, ext - 1024 * s)
                        scps = sc_pool.tile([128, 1024], F32, tag="sc")
                        for q5 in range(0, w, 512):
                            w5 = min(512, w - q5)
                            qabs = 128 * kt + 1024 * s + q5
                            nc.tensor.matmul(
                                scps[:, q5 : q5 + w5],
                                lhsT=k_sb[jt][hrow : hrow + 64, ts(kt, 128)],
                                rhs=q_sb[jt][hrow : hrow + 64, qabs : qabs + w5],
                                start=True,
                                stop=True,
                            )
                        nc.scalar.activation(
                            out=pt_t[:, 1024 * s : 1024 * s + w],
                            in_=scps[:, 0:w],
                            func=EXP,
                            scale=0.125,
                        )
                    # causal mask of the diagonal 128x128 block
                    nc.vector.tensor_mul(pt_t[:, 0:128], pt_t[:, 0:128], tri_sb)
                    for qb in range(kt // 4, 4):
                        lo = max(128 * kt, 512 * qb)
                        hi = 512 * (qb + 1)
                        rel = lo - 128 * kt
                        nc.tensor.matmul(
                            pvps[0:65, lo:hi],
                            lhsT=v_sb[kt][:, h, :],
                            rhs=pt_t[:, rel : rel + hi - lo],
                            start=(kt == 0),
                            stop=(kt == 4 * qb + 3),
                        )
                den_sb = rd_pool.tile([1, T], F32, tag="den")
                nc.vector.tensor_copy(out=den_sb, in_=pvps[64:65, :])
                rden = rd_pool.tile([1, T], F32, tag="rden")
                nc.vector.reciprocal_approx_fast(out=rden, in_=den_sb)
                dscr = d_pool.tile([T], F32, tag="dscr")
                nc.sync.dma_start(out=dscr.unsqueeze(0), in_=rden)
                rdb = rd_pool.tile([64, T], F32, tag="rdb")
                nc.sync.dma_start(out=rdb, in_=dscr.unsqueeze(0).to_broadcast([64, T]))
                nc.vector.tensor_tensor(
                    out=y_sb[jt][hrow : hrow + 64, :],
                    in0=pvps[0:64, :],
                    in1=rdb,
                    op=MULT,
                )

        # ---- Phase 3: output projection --------------------------------
        with (
            tc.tile_pool(name="osb", bufs=3) as o_pool,
            tc.tile_pool(name="ops", bufs=4, space="PSUM") as o_psum,
        ):
            for tt in range(TT):
                ot = o_pool.tile([128, C], F32, tag="o")
                for ch in range(2):
                    ps = o_psum.tile([128, 512], F32, tag="op")
                    for jt in range(4):
                        nc.tensor.matmul(
                            ps,
                            lhsT=_r(y_sb[jt][:, ts(tt, 128)]),
                            rhs=_r(wp_sb[:, jt, ts(ch, 512)]),
                            start=(jt == 0),
                            stop=(jt == 3),
                        )
                    nc.vector.tensor_tensor(
                        out=ot[:, ts(ch, 512)],
                        in0=ps,
                        in1=bp_sb[:, ts(ch, 512)],
                        op=ADD,
                    )
                nc.sync.dma_start(out=out[ts(tt, 128), :], in_=ot)


_CACHE = {}


def build_nc():
    if "nc" in _CACHE:
        return _CACHE["nc"]
    nc = bacc.Bacc(
        "TRN2",
        target_bir_lowering=False,
        debug=False,
        enable_asserts=False,
        num_devices=NCORES,
    )
    io = (
        nc.dram_tensor("xT", [C, T], F32R, kind="ExternalInput").ap(),
        nc.dram_tensor("wq", [C, JC], F32R, kind="ExternalInput").ap(),
        nc.dram_tensor("wk", [C, JC], F32R, kind="ExternalInput").ap(),
        nc.dram_tensor("wv", [C, JC], F32R, kind="ExternalInput").ap(),
        nc.dram_tensor("wp", [JC, C], F32R, kind="ExternalInput").ap(),
        nc.dram_tensor("bq", [JC], F32, kind="ExternalInput").ap(),
        nc.dram_tensor("bk", [JC], F32, kind="ExternalInput").ap(),
        nc.dram_tensor("bv", [JC], F32, kind="ExternalInput").ap(),
        nc.dram_tensor("bp", [C], F32, kind="ExternalInput").ap(),
        nc.dram_tensor("tri", [128, 128], BF16, kind="ExternalInput").ap(),
        nc.dram_tensor("out", [T, C], F32, kind="ExternalOutput").ap(),
    )
    with tile.TileContext(nc) as tc:
        _trace(nc, tc, io)
    nc.compile()
    _CACHE["nc"] = nc
    return nc


def make_in_maps(x, w_attn, b_attn, w_proj, b_proj):
    import ml_dtypes
    tri = np.triu(np.ones((128, 128), dtype=ml_dtypes.bfloat16))
    zeros_c = np.zeros(C, dtype=np.float32)
    in_maps = []
    for core in range(NCORES):
        b, hh = core // 2, core % 2
        j0 = JC * hh
        in_maps.append(
            {
                "xT": np.ascontiguousarray(x[b].T).astype(np.float32, copy=False),
                "wq": np.ascontiguousarray(w_attn[:, j0 : j0 + JC]),
                "wk": np.ascontiguousarray(w_attn[:, C + j0 : C + j0 + JC]),
                "wv": np.ascontiguousarray(w_attn[:, 2 * C + j0 : 2 * C + j0 + JC]),
                "wp": np.ascontiguousarray(w_proj[j0 : j0 + JC, :]),
                "bq": np.ascontiguousarray(b_attn[j0 : j0 + JC]),
                "bk": np.ascontiguousarray(b_attn[C + j0 : C + j0 + JC]),
                "bv": np.ascontiguousarray(b_attn[2 * C + j0 : 2 * C + j0 + JC]),
                "bp": (b_proj.astype(np.float32) if hh == 0 else zeros_c),
                "tri": tri,
            }
        )
    return in_maps


def gather(parts):
    out = np.empty((B, T, C), dtype=np.float32)
    for b in range(B):
        out[b] = parts[2 * b]["out"] + parts[2 * b + 1]["out"]
    return out


def kernel(x, w_attn, b_attn, w_proj, b_proj):
    x = np.asarray(x, dtype=np.float32)
    w_attn = np.asarray(w_attn, dtype=np.float32)
    b_attn = np.asarray(b_attn, dtype=np.float32)
    w_proj = np.asarray(w_proj, dtype=np.float32)
    b_proj = np.asarray(b_proj, dtype=np.float32)
    nc = build_nc()
    in_maps = make_in_maps(x, w_attn, b_attn, w_proj, b_proj)
    res = run_bass_kernel_spmd(nc, in_maps, core_ids=list(range(NCORES)))
    return gather(res.results)


if __name__ == "__main__":
    rng = np.random.default_rng(0)
    x = rng.standard_normal((B, T, C), dtype=np.float32)
    w_attn = rng.standard_normal((C, 3 * C), dtype=np.float32) / np.sqrt(C)
    b_attn = np.zeros(3 * C, np.float32)
    w_proj = rng.standard_normal((C, C), dtype=np.float32) / np.sqrt(C)
    b_proj = np.zeros(C, np.float32)
    out = kernel(x, w_attn, b_attn, w_proj, b_proj)
    print(out.shape, out.dtype, np.abs(out).mean())
